# revision 1
# baseline (speedup 1.0000x reference)
"""Trainium2 Bass kernel for NeuralRodriguesOperator.

Math:
  cos_t, sin_t = cos(theta), sin(theta)                       # (B, CJ)
  U   = W_bias  + einsum('jicpq,bc->bjipq', W_cos,  cos_t) + (W_sin,  sin_t)
  Ub  = Wb_bias + einsum('jicpq,bc->bjipq', Wb_cos, cos_t) + (Wb_sin, sin_t)
  out = einsum('bipq,bjiqr->bjpr', F_in, U) + einsum('bjipq,biqr->bjpr', Ub, F_in)

Device restructure (per core, C_L_out sharded 8 ways, JL=8 local j;
batch processed in 4 blocks of 128 on the partition dim):
  A-phase (PE, bf16): psA[b, (tc, jr)] = sum_k f[k, (sm,b)] * wm[k, (tc, jr)]
    where tc in {cos-c (16), sin-c (16)} = 32 blocks of jr=32 (2 PSUM
    banks); the bias product goes straight into psO (pre-zeroed by an
    ACT copy of a zero tile -- ACT may write PSUM).
  Evac (DVE): a1s[b, ts, tc, jr] = psA * tb[b, tc]  (tb = [cos_t | sin_t]
    per batch-block; the per-(b,c) trig scale is fused into the PSUM
    drain via a broadcast tensor_tensor, so the c-contraction becomes a
    plain sum).
  Cred (PE): psO[b, v, (ts, jr)] += I @ a1s[:, :, 2u+v, :] for u in
    0..15 — 16 paired identity-stationary matmuls of N=512 (pairs halve
    the instruction/LoadStationary count; v-halves summed at the end).
  Final (ACT+Pool): copy psO out, sum v-halves, out[b,p,j,r] =
    term1 + transpose_pr(term2); DMA out.  Trig range reduction and the
    final adds run on Pool so DVE stays dedicated to evacuation.

All weights/F in bf16: f32r moving operands stream far below peak on
real HW (measured ~4-5x slower than the cost model), bf16 runs at
1 col/cycle.  Accumulation stays f32 in PSUM.  Cred for block bb is
emitted after the A-phase of bb+1 so the PE never waits on the PSUM
drain.  Weight DMA is issued inside the body with double-buffered tiles
so iteration i+1's loads overlap i's compute under KERNEL_REPS, and
theta is issued first so the trig chain at the head of the in-order DVE
queue never blocks the first evacuation.
"""

import os

import ml_dtypes
import numpy as np

import concourse.bacc as bacc
import concourse.bass as bass
import concourse.mybir as mybir
from concourse.bass_utils import run_bass_kernel_spmd
from concourse.masks import make_identity
from concourse.tile import TileContext

B = 512
CI = 64  # C_L_in
CO = 64  # C_L_out
CJ = 16
NCORES = 8
JL = CO // NCORES  # 8 j per core
K = CI * 4  # contraction (i,q) = 256
PB = 4 * B  # (p,b) columns = 2048
NBB = 4  # b blocks of 128
NTC = 2 * CJ + 1  # cos-c, sin-c, bias blocks
JR = JL * 4  # 32
PI = float(np.pi)

F32 = mybir.dt.float32
BF16 = mybir.dt.bfloat16

last_exec_time_ns = None
last_results = None

_nc_cache = {}


def _body(nc, tc, wpool, apool, opool, spool, psA, psO, identity,
          zbias, zero512, dram_in, out_d):
    mm = nc.tensor.matmul

    # ---- theta first (host ships it [128, NBB, CJ]: one DMA, so the
    # trig chain that gates the first evacuation starts ~1.7us earlier) ----
    tht = spool.tile([128, NBB, CJ], F32, tag="tht", name="tht")
    nc.sync.dma_start(tht, dram_in["theta"][:, :])

    # ---- weight / F loads (double-buffered pool; first-use order) ----
    fk = []  # fk[t][k] : [128, 2048] bf16
    wm = []  # wm[t][k] : [128, 1056] bf16
    for t in range(2):
        fk.append([wpool.tile([128, PB], BF16, tag=f"f{t}k{k}", name=f"f{t}k{k}")
                   for k in range(2)])
        wm.append([wpool.tile([128, NTC * JR], BF16, tag=f"w{t}k{k}", name=f"w{t}k{k}")
                   for k in range(2)])
    # F tiles split into per-sm column chunks, issued in first-use order so
    # the first A matmuls start after ~0.4MB instead of ~2.1MB of DMA.
    for t, (fd, wd) in enumerate((("f1t", "wm1"), ("f2t", "wm2"))):
        for k in range(2):
            nc.sync.dma_start(
                fk[t][k][:, 0:B],
                dram_in[fd][k * 128:(k + 1) * 128, 0:B],
            )
            nc.sync.dma_start(wm[t][k], dram_in[wd][k * 128:(k + 1) * 128, :])
    for sm in range(1, 4):
        for t, fd in enumerate(("f1t", "f2t")):
            for k in range(2):
                nc.sync.dma_start(
                    fk[t][k][:, sm * B:(sm + 1) * B],
                    dram_in[fd][k * 128:(k + 1) * 128, sm * B:(sm + 1) * B],
                )

    # ---- trig: theta -> cos/sin with range reduction, all bb at once ----
    x = spool.tile([128, 2, NBB, CJ], F32, tag="x", name="x")
    nc.gpsimd.tensor_scalar_add(x[:, 0], tht, PI / 2)
    nc.gpsimd.tensor_copy(x[:, 1], tht)
    g = spool.tile([128, 2, NBB, CJ], F32, tag="g", name="g")
    l = spool.tile([128, 2, NBB, CJ], F32, tag="l", name="l")
    nc.gpsimd.tensor_scalar(g, x, PI, None, mybir.AluOpType.is_gt)
    nc.gpsimd.tensor_scalar(l, x, -PI, None, mybir.AluOpType.is_lt)
    nc.gpsimd.tensor_sub(g, g, l)
    nc.gpsimd.tensor_scalar_mul(g, g, 2.0 * PI)
    nc.gpsimd.tensor_sub(x, x, g)  # now in [-pi, pi]
    trig = spool.tile([128, 2, NBB, CJ], F32, tag="trig", name="trig")
    nc.scalar.activation(trig, x, mybir.ActivationFunctionType.Sin, bias=zbias)

    # tb_all[:, bb] : [128, 2*CJ, 1] = [cos_t(16) | sin_t(16)]
    tb_all = spool.tile([128, NBB, 2 * CJ, 1], F32, tag="tb", name="tb")
    nc.gpsimd.tensor_copy(tb_all[:, :, 0:CJ, 0], trig[:, 0])
    nc.gpsimd.tensor_copy(tb_all[:, :, CJ:2 * CJ, 0], trig[:, 1])
    tbs = [tb_all[:, bb] for bb in range(NBB)]

    # ---- main pipeline over b blocks ----
    def a_phase(bb):
        """A-phase: trig-block matmuls into psA (2 banks), bias matmuls
        directly into psO, DVE fused-scale evacuations into a1s.

        psO is pre-zeroed by an ACT copy (all bias/cred matmuls then
        accumulate with start=False; per-slice start=True would clobber
        via region pending-zero).
        """
        a1s = apool.tile([128, 8, 2 * CJ, JR], BF16, tag="a1s", name="a1s")
        # psO dims (v, ts, jr): v = tc parity; halves summed on DVE at the
        # end.  One full bank -> paired N=512 cred matmuls (half the
        # instruction count / stationary loads).
        ops = psO.tile([128, 2, 8, JR], F32, tag="psO", name="psO")
        opsv = ops
        nc.scalar.copy(opsv, zero512)
        for t in range(2):
            for sm in range(4):
                ts = t * 4 + sm
                off = sm * B + bb * 128
                ps = psA.tile([128, 2 * CJ, JR], F32, tag="psA", name="psA")
                # k-alternating order: back-to-back matmuls with an
                # identical stationary serialize the reload (~+80ns/mm
                # measured); alternating k-chunks hides it entirely.
                lhs = [fk[t][k][:, off:off + 128] for k in range(2)]
                w = wm[t]
                for c0, c1 in ((0, CJ * JR), (CJ * JR, 2 * CJ * JR)):
                    rg = ps[:, c0 // JR:c1 // JR]
                    mm(rg, lhs[0], w[0][:, c0:c1], start=True, stop=False)
                    mm(rg, lhs[1], w[1][:, c0:c1], start=False, stop=True)
                for k in range(2):
                    mm(opsv[:, 0, ts], lhs[k], w[k][:, 2 * CJ * JR:],
                       start=False, stop=False, skip_group_check=True)
                # fused evacuation: a1s[:, ts] = psA * tb  (broadcast over jr)
                dst = a1s[:, ts]
                if sm == 3:
                    # relieve DVE: ACT drains PSUM, Pool applies the scale
                    tmp = opool.tile([128, 2 * CJ, JR], F32, tag=f"tmp{t}",
                                     name=f"tmp{t}")
                    nc.scalar.copy(tmp, ps)
                    in0, in1 = bass.broadcast_tensor_aps(
                        tmp[:, :, :], tbs[bb][:, :, :]
                    )
                    nc.gpsimd.tensor_tensor(dst, in0, in1, mybir.AluOpType.mult)
                else:
                    in0, in1 = bass.broadcast_tensor_aps(
                        ps[:, :, :], tbs[bb][:, :, :]
                    )
                    nc.vector.tensor_tensor(dst, in0, in1, mybir.AluOpType.mult)
        return a1s, ops, opsv

    def cred_phase(bb, a1s, ops, opsv):
        """c-contraction: 16 paired identity matmuls; final combine; store."""
        for u in range(CJ):
            pair = a1s[:, :, 2 * u:2 * u + 2, :].rearrange(
                "n s v j -> n v s j")
            mm(opsv, identity[u % 2], pair,
               start=False, stop=u == CJ - 1, skip_group_check=True)
        osum = opool.tile([128, 2, 2, 4, JL, 4], F32, tag="osum", name="osum")
        nc.scalar.copy(osum, ops)
        s = opool.tile([128, 2, 4, JL, 4], F32, tag="s", name="s")
        nc.gpsimd.tensor_add(s, osum[:, 0], osum[:, 1])
        osb = opool.tile([128, 4, JL, 4], F32, tag="osb", name="osb")
        nc.gpsimd.tensor_add(osb, s[:, 0],
                             s[:, 1].rearrange("n r j p -> n p j r"))
        nc.sync.dma_start(out_d[bb * 128:(bb + 1) * 128], osb)

    prev = None
    for bb in range(NBB):
        cur = a_phase(bb)
        if prev is not None:
            cred_phase(bb - 1, *prev)
        prev = cur
    cred_phase(NBB - 1, *prev)


def _build_bass(reps=1):
    nc = bacc.Bacc(None)
    dram_in = {}
    for name, shape, dt in [
        ("f1t", [K, PB], BF16),  # F[b,i,p,q] -> [(i,q),(p,b)]
        ("f2t", [K, PB], BF16),  # F[b,i,q,r] -> [(i,q),(r,b)]
        ("wm1", [K, NTC * JR], BF16),  # [cos|sin|bias] [(i,q),(tc,j,r)]
        ("wm2", [K, NTC * JR], BF16),
        ("theta", [128, NBB * CJ], F32),
    ]:
        dram_in[name] = nc.declare_dram_parameter(name, shape, dt, isOutput=False)
    out_d = nc.declare_dram_parameter("out", [B, 4, JL, 4], F32, isOutput=True)

    with TileContext(nc) as tc:
        with (
            tc.tile_pool(name="consts", bufs=1) as consts,
            tc.tile_pool(name="w", bufs=2) as wpool,
            tc.tile_pool(name="a1s", bufs=2) as apool,
            tc.tile_pool(name="osb", bufs=2) as opool,
            tc.tile_pool(name="scratch", bufs=2) as spool,
            tc.tile_pool(name="psA", bufs=3, space="PSUM") as psA,
            tc.tile_pool(name="psO", bufs=2, space="PSUM") as psO,
        ):
            # two copies: alternating stationaries avoid the
            # serialized reload of back-to-back identical weights
            identity = [consts.tile([128, 128], BF16, name=f"ident{i}")
                        for i in range(2)]
            make_identity(nc, identity[0])
            nc.gpsimd.tensor_copy(identity[1], identity[0])
            zbias = consts.tile([128, 1], F32)
            nc.vector.memset(zbias, 0.0)
            zero512 = consts.tile([128, 512], BF16)
            nc.vector.memset(zero512, 0.0)

            if reps > 1:
                with tc.For_i(0, reps, 1):
                    _body(nc, tc, wpool, apool, opool, spool, psA, psO,
                          identity, zbias, zero512, dram_in, out_d)
            else:
                _body(nc, tc, wpool, apool, opool, spool, psA, psO,
                      identity, zbias, zero512, dram_in, out_d)
    nc.compile()
    return nc


def _host_prep(F_in, theta, W_bias, W_cos, W_sin, Wb_bias, Wb_cos, Wb_sin):
    """Layout-only host prep (no arithmetic). Returns per-core input maps."""
    bf16 = ml_dtypes.bfloat16
    f = np.asarray(F_in, dtype=np.float32)
    # [(i,q), (p,b)]
    f1t = np.ascontiguousarray(
        np.transpose(f, (1, 3, 2, 0)).reshape(K, PB)).astype(bf16)
    # [(i,q), (r,b)]
    f2t = np.ascontiguousarray(
        np.transpose(f, (1, 2, 3, 0)).reshape(K, PB)).astype(bf16)
    # [128, (bb, c)]: partition-major layout for a single DMA
    th = np.ascontiguousarray(
        np.asarray(theta, dtype=np.float32).reshape(NBB, 128, CJ)
        .transpose(1, 0, 2).reshape(128, NBB * CJ))

    in_maps = []
    for core in range(NCORES):
        js = slice(core * JL, (core + 1) * JL)
        # W_cos/W_sin [j,i,c,p,q] -> [(i,p),(c,j,q)] (term1 K=(i,p), out q)
        wc1 = np.transpose(np.asarray(W_cos)[js], (1, 3, 2, 0, 4)).reshape(K, -1)
        ws1 = np.transpose(np.asarray(W_sin)[js], (1, 3, 2, 0, 4)).reshape(K, -1)
        wb1 = np.transpose(np.asarray(W_bias)[js], (1, 2, 0, 3)).reshape(K, -1)
        # Wb_* [j,i,c,p,q] -> [(i,q),(c,j,p)] (term2 K=(i,q), out p)
        wc2 = np.transpose(np.asarray(Wb_cos)[js], (1, 4, 2, 0, 3)).reshape(K, -1)
        ws2 = np.transpose(np.asarray(Wb_sin)[js], (1, 4, 2, 0, 3)).reshape(K, -1)
        wb2 = np.transpose(np.asarray(Wb_bias)[js], (1, 3, 0, 2)).reshape(K, -1)
        wm1 = np.ascontiguousarray(
            np.concatenate([wc1, ws1, wb1], axis=1)).astype(bf16)
        wm2 = np.ascontiguousarray(
            np.concatenate([wc2, ws2, wb2], axis=1)).astype(bf16)
        in_maps.append(
            {"f1t": f1t, "f2t": f2t, "wm1": wm1, "wm2": wm2, "theta": th}
        )
    return in_maps


_prep_cache = {}


def kernel(F_in, theta, W_bias, W_cos, W_sin, Wb_bias, Wb_cos, Wb_sin):
    global _nc_cache, last_exec_time_ns, last_results
    reps = int(os.environ.get("KERNEL_REPS", "1"))
    if reps not in _nc_cache:
        _nc_cache[reps] = _build_bass(reps=reps)
    nc = _nc_cache[reps]

    pkey = id(F_in)
    if pkey not in _prep_cache:
        _prep_cache.clear()
        _prep_cache[pkey] = _host_prep(
            F_in, theta, W_bias, W_cos, W_sin, Wb_bias, Wb_cos, Wb_sin
        )
    in_maps = _prep_cache[pkey]
    res = run_bass_kernel_spmd(nc, in_maps, core_ids=list(range(NCORES)))
    last_exec_time_ns = res.exec_time_ns
    last_results = res

    # gather: core j-slab [b, (p, j_local, r)] -> [b, j_local, p, r]
    out = np.empty((B, CO, 4, 4), dtype=np.float32)
    for core in range(NCORES):
        co = res.results[core]["out"].reshape(B, 4, JL, 4)
        out[:, core * JL:(core + 1) * JL] = np.transpose(co, (0, 2, 1, 3))
    return out

